# revision 7
# baseline (speedup 1.0000x reference)
"""Trainium2 Bass kernel for nn_Discriminator (HouseGAN-style GNN discriminator).

Strategy: node-parallel over 8 NeuronCores (8 nodes/core).  All convs are
TensorE matmuls over a node-major layout [8 nodes x C channels, padded px],
3x3 taps realized as 9 PSUM-accumulated matmuls with shifted rhs APs
(float32r).  Message passing = AllGather of bf16 features + dense A (X) I_16
matmuls built host-side from the edge list.  Graph-level segment sums of the
head outputs are index-only work done on host.

Self-contained: hardcodes V=64, E=256, B=8, HW=64, 8 cores.
"""
import numpy as np
import ml_dtypes

V, E, B = 64, 256, 8
HW = 64
N_CORES = 8
BF16 = ml_dtypes.bfloat16

# ---------------------------------------------------------------------------
# layout constants
# ---------------------------------------------------------------------------


def res_params(R):
    W = R + 2
    G = W + 2
    PX = R * W
    return W, G, PX


def L_of(R):
    W, G, PX = res_params(R)
    return G + PX + G


DEC_LEAD = 16
NS1 = 100            # dec_in node stride (8x10 image + 2 zero rows)
D1_LEAD = 8
NS2 = 36             # d1 node stride (4x6 + 2 zero rows)
NS3 = 16             # d2/d3 node stride (2x4 + 2 zero rows)
DEC_IN_L = DEC_LEAD + 8 * NS1 + 16
D1_L = D1_LEAD + 8 * NS2 + 8
D23_L = D1_LEAD + 8 * NS3 + 8

# ---------------------------------------------------------------------------
# host prep
# ---------------------------------------------------------------------------


def pad_images(x, R):
    W, G, PX = res_params(R)
    lead = x.shape[:-2]
    out = np.zeros(lead + (G + PX + G,), dtype=np.float32)
    xpad = np.zeros(lead + (R, W), dtype=np.float32)
    xpad[..., :R] = x
    out[..., G:G + PX] = xpad.reshape(lead + (PX,))
    return out


def conv_stats_flat(Wt, n_nodes, k_stride, m_stride, K=128, M=128):
    """[C_out, C_in, 3, 3] -> [K, 9*M] flat block-diag stationaries."""
    C_out, C_in = Wt.shape[:2]
    out = np.zeros((K, 9 * M), dtype=np.float32)
    for t in range(9):
        blk = Wt[:, :, t // 3, t % 3].T  # [C_in, C_out]
        for n in range(n_nodes):
            out[n * k_stride:n * k_stride + C_in,
                t * M + n * m_stride:t * M + n * m_stride + C_out] = blk
    return out


def bias_vec(b, n_nodes, stride):
    out = np.zeros((128, 1), dtype=np.float32)
    for n in range(n_nodes):
        out[n * stride:n * stride + b.shape[0], 0] = b
    return out


def build_adjacency(given_w):
    Apos = np.zeros((V, V), dtype=np.float32)
    Aneg = np.zeros((V, V), dtype=np.float32)
    gw = np.asarray(given_w).astype(np.int64)
    for e in range(gw.shape[0]):
        s, sg, d = int(gw[e, 0]), int(gw[e, 1]), int(gw[e, 2])
        if sg > 0:
            Apos[d, s] += 1.0
            Apos[s, d] += 1.0
        elif sg < 0:
            Aneg[d, s] += 1.0
            Aneg[s, d] += 1.0
    return Apos, Aneg


def amix_stats_flat(A, core):
    """[V,V] counts -> [128, 8*128] flat: cols j*128 + (16 d_lo + c)."""
    S = np.zeros((128, 8 * 128), dtype=np.float32)
    I16 = np.eye(16, dtype=np.float32)
    for j in range(8):
        for s_lo in range(8):
            for d_lo in range(8):
                a = A[8 * core + d_lo, 8 * j + s_lo]
                if a != 0.0:
                    S[16 * s_lo:16 * s_lo + 16,
                      j * 128 + 16 * d_lo:j * 128 + 16 * d_lo + 16] = a * I16
    return S.astype(BF16)


def prep_host(x, given_y, given_w, nd_to_sample, params):
    x = np.asarray(x, np.float32).reshape(V, HW, HW)
    given_y = np.asarray(given_y, np.float32)
    nd = np.asarray(nd_to_sample).astype(np.int64)
    p = params

    def n32(a):
        return np.asarray(a, np.float32)

    Apos, Aneg = build_adjacency(given_w)

    W64, G64, PX64 = res_params(64)
    l1_w = n32(p['l1_w']).reshape(8, HW, HW, 18)
    l1_b = n32(p['l1_b']).reshape(8, HW, HW)
    l1_rhs = np.zeros((19, 8 * PX64), dtype=np.float32)
    for c in range(8):
        wpad = np.zeros((HW, W64, 18), dtype=np.float32)
        wpad[:, :HW] = l1_w[c]
        l1_rhs[:18, c * PX64:(c + 1) * PX64] = wpad.reshape(PX64, 18).T
        bpad = np.zeros((HW, W64), dtype=np.float32)
        bpad[:, :HW] = l1_b[c]
        l1_rhs[18, c * PX64:(c + 1) * PX64] = bpad.reshape(PX64)

    shared = {'l1_rhs': l1_rhs.astype(BF16)}
    for i, (w, b) in enumerate(p['encoder']):
        shared[f'stat_enc{i}'] = conv_stats_flat(n32(w), 8, 16, 16).astype(BF16)
        shared[f'bias_enc{i}'] = bias_vec(n32(b), 8, 16)
    for si, cmp in enumerate((p['cmp1'], p['cmp2'], p['cmp3'])):
        (w1, b1), (w2, b2), (w3, b3) = cmp
        w1 = n32(w1)
        # duplicated in both partition halves: walrus requires fmap and
        # weight to share the SBUF base partition
        shared[f'cmp{si}_sf'] = np.vstack([
            conv_stats_flat(w1[:, 0:16], 4, 16, 32, K=64)] * 2).astype(BF16)
        shared[f'cmp{si}_sp'] = np.vstack([
            conv_stats_flat(w1[:, 16:32], 4, 16, 32, K=64)] * 2).astype(BF16)
        shared[f'cmp{si}_sn'] = np.vstack([
            conv_stats_flat(w1[:, 32:48], 4, 16, 32, K=64)] * 2).astype(BF16)
        shared[f'cmp{si}_s2'] = conv_stats_flat(n32(w2), 4, 32, 32).astype(BF16)
        shared[f'cmp{si}_s3'] = conv_stats_flat(n32(w3), 4, 32, 16, M=64).astype(BF16)
        shared[f'cmp{si}_b1'] = bias_vec(n32(b1), 4, 32)
        shared[f'cmp{si}_b2'] = bias_vec(n32(b2), 4, 32)
        shared[f'cmp{si}_b3'] = bias_vec(n32(b3), 8, 16)
    for i, (w, b) in enumerate((p['ds1'], p['ds2'], p['ds3'])):
        shared[f'stat_ds{i}'] = conv_stats_flat(n32(w), 8, 16, 16).astype(BF16)
        shared[f'bias_ds{i}'] = bias_vec(n32(b), 8, 16)

    dw1, db1 = n32(p['decoder'][0][0]), n32(p['decoder'][0][1])
    dw2, db2 = n32(p['decoder'][1][0]), n32(p['decoder'][1][1])
    dw3, db3 = n32(p['decoder'][2][0]), n32(p['decoder'][2][1])
    sd1 = np.zeros((16, 2 * 9 * 128), dtype=np.float32)
    sd2 = np.zeros((128, 2 * 9 * 128), dtype=np.float32)
    for h in range(2):
        for t in range(9):
            sd1[:, (h * 9 + t) * 128:(h * 9 + t + 1) * 128] = \
                dw1[128 * h:128 * h + 128, :, t // 3, t % 3].T
            sd2[:, (h * 9 + t) * 128:(h * 9 + t + 1) * 128] = \
                dw2[:, 128 * h:128 * h + 128, t // 3, t % 3].T
    sd3 = np.zeros((128, 9 * 128), dtype=np.float32)
    for t in range(9):
        sd3[:, t * 128:(t + 1) * 128] = dw3[:, :, t // 3, t % 3].T
    shared['stat_dec1'] = sd1.astype(BF16)
    shared['stat_dec2'] = sd2.astype(BF16)
    shared['stat_dec3'] = sd3.astype(BF16)
    shared['bias_dec1'] = np.stack([db1[0:128], db1[128:256]], axis=1)
    shared['bias_dec2'] = db2.reshape(128, 1)
    shared['bias_dec3'] = db3.reshape(128, 1)

    wc = (n32(p['fcg_w']) + 5.0 * n32(p['fcl_w'])).reshape(128)
    hs = np.zeros((128, 16), dtype=np.float32)
    for hw in range(4):
        for bblk in range(4):
            for c_lo in range(32):
                hs[32 * bblk + c_lo, hw * 4 + bblk] = wc[c_lo * 4 + hw]
    shared['head_stat'] = hs.astype(BF16)

    in_maps = []
    for core in range(N_CORES):
        xcat = np.zeros((128, L_of(64)), dtype=np.float32)
        for n in range(8):
            xcat[16 * n] = pad_images(x[8 * core + n], 64)
        gyT = np.zeros((19, 8), dtype=np.float32)
        gyT[:18] = given_y[8 * core:8 * core + 8].T
        gyT[18] = 1.0
        m = dict(shared)
        m['xcat'] = xcat.astype(BF16)
        m['gyT'] = gyT.astype(BF16)
        m['amix_pos'] = amix_stats_flat(Apos, core)
        m['amix_neg'] = amix_stats_flat(Aneg, core)
        in_maps.append(m)

    host_ctx = dict(nd=nd,
                    fcg_b=float(np.asarray(p['fcg_b']).reshape(-1)[0]),
                    fcl_b=float(np.asarray(p['fcl_b']).reshape(-1)[0]))
    return in_maps, host_ctx


def finish_host(head_parts, host_ctx):
    nd = host_ctx['nd']
    s = np.zeros(4 * V, dtype=np.float64)
    for core in range(N_CORES):
        hp = np.asarray(head_parts[core], np.float64)  # [4, 8]
        for v_lo in range(8):
            for b_ in range(4):
                s[4 * (8 * core + v_lo) + b_] = hp[b_, v_lo]
    out = np.zeros((B, 1), dtype=np.float64)
    counts = np.zeros(B, dtype=np.float64)
    for r in range(4 * V):
        out[int(nd[r]), 0] += s[r]
        counts[int(nd[r])] += 1.0
    out[:, 0] += host_ctx['fcg_b'] + 5.0 * counts * host_ctx['fcl_b']
    return out.astype(np.float32)


# ---------------------------------------------------------------------------
# bass program
# ---------------------------------------------------------------------------


def _input_specs():
    sp = {
        'xcat': ([128, L_of(64)], 'bf16'),
        'gyT': ([19, 8], 'bf16'),
        'l1_rhs': ([19, 8 * 4224], 'bf16'),
        'amix_pos': ([128, 1024], 'bf16'),
        'amix_neg': ([128, 1024], 'bf16'),
        'stat_dec1': ([16, 2304], 'bf16'),
        'stat_dec2': ([128, 2304], 'bf16'),
        'stat_dec3': ([128, 1152], 'bf16'),
        'bias_dec1': ([128, 2], 'f32'),
        'bias_dec2': ([128, 1], 'f32'),
        'bias_dec3': ([128, 1], 'f32'),
        'head_stat': ([128, 16], 'bf16'),
    }
    for i in range(4):
        sp[f'stat_enc{i}'] = ([128, 1152], 'bf16')
        sp[f'bias_enc{i}'] = ([128, 1], 'f32')
    for s in range(3):
        sp[f'cmp{s}_sf'] = ([128, 1152], 'bf16')
        sp[f'cmp{s}_sp'] = ([128, 1152], 'bf16')
        sp[f'cmp{s}_sn'] = ([128, 1152], 'bf16')
        sp[f'cmp{s}_s2'] = ([128, 1152], 'bf16')
        sp[f'cmp{s}_s3'] = ([128, 576], 'bf16')
        sp[f'cmp{s}_b1'] = ([128, 1], 'f32')
        sp[f'cmp{s}_b2'] = ([128, 1], 'f32')
        sp[f'cmp{s}_b3'] = ([128, 1], 'f32')
        sp[f'stat_ds{s}'] = ([128, 1152], 'bf16')
        sp[f'bias_ds{s}'] = ([128, 1], 'f32')
    return sp


def px_chunks(PX, maxn=512):
    out = []
    c = 0
    while c < PX:
        n = min(maxn, PX - c)
        out.append((c, n))
        c += n
    return out


def build_program(debug_taps=()):
    import concourse.bass as bass
    import concourse.bacc as bacc
    import concourse.tile as tile
    import concourse.mybir as mybir
    import contextlib

    f32 = mybir.dt.float32
    f32r = mybir.dt.float32r
    bf16 = mybir.dt.bfloat16
    LRELU = mybir.ActivationFunctionType.Prelu

    nc = bacc.Bacc("TRN2", target_bir_lowering=False, debug=False,
                   num_devices=N_CORES)

    dt_map = {'f32': f32, 'bf16': bf16}
    ins = {}
    for name, (shape, dt) in _input_specs().items():
        ins[name] = nc.dram_tensor(name, shape, dt_map[dt],
                                   kind="ExternalInput").ap()
    head_d = nc.dram_tensor("head_part", [4, 8], f32,
                            kind="ExternalOutput").ap()
    dbg = {}
    for name in debug_taps:
        dbg[name] = nc.dram_tensor(f"dbg_{name}", [128, L_of(64)], bf16,
                                   kind="ExternalOutput").ap()

    cin = {}
    cout = {}
    for s, R in enumerate((64, 32, 16)):
        cin[s] = nc.dram_tensor(f"cin{s}", [128, L_of(R)], bf16,
                                kind="Internal").ap()
        cout[s] = nc.dram_tensor(f"cout{s}", [8, 128, L_of(R)], bf16,
                                 kind="Internal", addr_space="Shared").ap()

    def dbg_dump(name, t):
        if name in dbg:
            w = t[:].ap[-1][1]
            nc.sync.dma_start(out=dbg[name][0:t[:].ap[0][1], 0:w], in_=t[:])

    with tile.TileContext(nc) as tc:
        ctx = contextlib.ExitStack()
        with ctx:
            acts = ctx.enter_context(tc.tile_pool(name="acts", bufs=5))
            iop = ctx.enter_context(tc.tile_pool(name="iop", bufs=1))
            fpool = ctx.enter_context(tc.tile_pool(name="fp", bufs=2))
            wpool = ctx.enter_context(tc.tile_pool(name="w", bufs=3))
            spool = ctx.enter_context(tc.tile_pool(name="small", bufs=4))
            slabp = ctx.enter_context(tc.tile_pool(name="slabp", bufs=2))
            pspool = ctx.enter_context(tc.tile_pool(name="ps", bufs=4,
                                                    space="PSUM"))
            pshp = ctx.enter_context(tc.tile_pool(name="psh", bufs=1,
                                                  space="PSUM"))
            consts = ctx.enter_context(tc.tile_pool(name="consts", bufs=1))

            def load_w(name):
                shape, _ = _input_specs()[name]
                t = wpool.tile(shape, bf16, tag="w1152")
                nc.sync.dma_start(out=t[:], in_=ins[name])
                return t

            def load_bias(name):
                shape, _ = _input_specs()[name]
                t = spool.tile(shape, f32, tag="bias")
                nc.sync.dma_start(out=t[:], in_=ins[name])
                return t

            def fresh_tile(pool, R, tag):
                """fresh [128, L_of(R)] bf16 tile with guards zeroed."""
                W, G, PX = res_params(R)
                t = pool.tile([128, L_of(R)], bf16, tag=tag)
                nc.vector.memset(t[:, 0:G], 0.0)
                nc.vector.memset(t[:, G + PX:], 0.0)
                return t

            def zero_padcols(t, R, M=128):
                W, G, PX = res_params(R)
                tap = t[:]
                ap = bass.AP(tensor=tap.tensor, offset=tap.offset + G + R,
                             ap=[[tap.ap[0][0], M], [W, R], [1, 2]])
                nc.vector.memset(ap, 0.0)

            def emit_conv(parts, dst, R, bias_t, bias_col=0):
                """parts: (src, k_base, K, stat, col0, M, m_base) list;
                single psum region accumulation per px chunk + Lrelu evict."""
                W, G, PX = res_params(R)
                for (c0, n) in px_chunks(PX):
                    ps = pspool.tile([128, 512], f32, tag="ps")
                    total = len(parts) * 9
                    i = 0
                    for (src, k_base, K, stat, col0, M, m_base) in parts:
                        for t in range(9):
                            off = (t // 3 - 1) * W + (t % 3 - 1)
                            rhs = src[k_base:k_base + K,
                                      G + c0 + off:G + c0 + off + n]
                            lhsT = stat[k_base:k_base + K,
                                        col0 + t * M:col0 + t * M + M]
                            i += 1
                            nc.tensor.matmul(
                                ps[m_base:m_base + M, 0:n],
                                lhsT, rhs,
                                start=(i == 1), stop=(i == total),
                                tile_position=(k_base, m_base))
                    nc.scalar.activation(
                        out=dst[:, G + c0:G + c0 + n], in_=ps[:, 0:n],
                        func=LRELU, bias=bias_t[:, bias_col:bias_col + 1],
                        alpha=0.1)
                zero_padcols(dst, R)

            def emit_ds(src, dst, R, stat, bias_t):
                W, G, PX = res_params(R)
                Ro = R // 2
                Wo, Go, PXo = res_params(Ro)
                r0 = 0
                while r0 < Ro:
                    nr = min(15, Ro - r0)
                    ncols = nr * Wo
                    ps = pspool.tile([128, 512], f32, tag="ps")
                    sap = src[:]
                    for t in range(9):
                        dr, dc = t // 3 - 1, t % 3 - 1
                        base = G + (2 * r0 + dr) * W + dc
                        rhs = bass.AP(
                            tensor=sap.tensor, offset=sap.offset + base,
                            ap=[[sap.ap[0][0], 128], [2 * W, nr], [2, Wo]])
                        lhsT = stat[0:128, t * 128:t * 128 + 128]
                        nc.tensor.matmul(ps[0:128, 0:ncols],
                                         lhsT, rhs,
                                         start=(t == 0), stop=(t == 8))
                    nc.scalar.activation(
                        out=dst[:, Go + r0 * Wo:Go + r0 * Wo + ncols],
                        in_=ps[:, 0:ncols], func=LRELU,
                        bias=bias_t[:, 0:1], alpha=0.1)
                    r0 += nr
                zero_padcols(dst, Ro)

            # ---------------- l1 + xcat assembly ----------------
            xcat_t = acts.tile([128, L_of(64)], bf16, tag="act")
            nc.sync.dma_start(out=xcat_t[:], in_=ins['xcat'])
            gyT_t = consts.tile([19, 8], bf16)
            nc.sync.dma_start(out=gyT_t[:], in_=ins['gyT'])
            W64, G64, PX64 = res_params(64)
            xpitch = xcat_t[:].ap[0][0]
            with tc.tile_pool(name="l1", bufs=2) as l1p:
                QW = 1056  # quarter channel per iteration
                for ch in range(32):
                    c, half = ch // 4, ch % 4
                    rhs_t = l1p.tile([19, QW], bf16, tag="l1rhs")
                    nc.sync.dma_start(
                        out=rhs_t[:],
                        in_=ins['l1_rhs'][:, c * 4224 + half * QW:
                                          c * 4224 + (half + 1) * QW])
                    y_t = l1p.tile([8, QW], bf16, tag="l1y")
                    for (c0, n) in px_chunks(QW):
                        ps = pspool.tile([128, 512], f32, tag="ps")
                        nc.tensor.matmul(ps[0:8, 0:n], gyT_t[:],
                                         rhs_t[:, c0:c0 + n],
                                         start=True, stop=True)
                        nc.vector.tensor_copy(y_t[:, c0:c0 + n], ps[0:8, 0:n])
                    yap = y_t[:]
                    dst = bass.AP(
                        tensor=xcat_t[:].tensor,
                        offset=xcat_t[:].offset + (1 + c) * xpitch + G64
                        + half * QW,
                        ap=[[16 * xpitch, 8], [1, QW]])
                    nc.sync.dma_start(out=dst, in_=yap)

            # ---------------- encoder ----------------
            src = xcat_t
            for i in range(4):
                st = load_w(f'stat_enc{i}')
                bi = load_bias(f'bias_enc{i}')
                dst = fresh_tile(acts, 64, "act")
                emit_conv([(src, 0, 128, st, 0, 128, 0)], dst, 64, bi)
                src = dst
            f_t = src
            dbg_dump('f64', f_t)

            amix_pos_t = consts.tile([128, 1024], bf16)
            nc.sync.dma_start(out=amix_pos_t[:], in_=ins['amix_pos'])
            amix_neg_t = consts.tile([128, 1024], bf16)
            nc.sync.dma_start(out=amix_neg_t[:], in_=ins['amix_neg'])

            # ---------------- cmp stages ----------------
            for s, R in enumerate((64, 32, 16)):
                W, G, PX = res_params(R)
                L = L_of(R)
                nc.sync.dma_start(out=cin[s], in_=f_t[:])
                nc.gpsimd.collective_compute(
                    "AllGather", mybir.AluOpType.bypass,
                    ins=[cin[s]], outs=[cout[s]],
                    replica_groups=[list(range(N_CORES))])
                pos_t = fresh_tile(iop, R, "pos")
                neg_t = fresh_tile(iop, R, "neg")
                for (c0, n) in px_chunks(PX):
                    slab = slabp.tile([128, 8, 512], bf16, tag="slab")
                    for j in range(8):
                        nc.sync.dma_start(
                            out=slab[:, j, 0:n],
                            in_=cout[s][j, :, G + c0:G + c0 + n])
                    psp = pspool.tile([128, 512], f32, tag="ps")
                    psn = pspool.tile([128, 512], f32, tag="ps")
                    for j in range(8):
                        nc.tensor.matmul(psp[:, 0:n],
                                         amix_pos_t[:, j * 128:(j + 1) * 128],
                                         slab[:, j, 0:n],
                                         start=(j == 0), stop=(j == 7))
                    for j in range(8):
                        nc.tensor.matmul(psn[:, 0:n],
                                         amix_neg_t[:, j * 128:(j + 1) * 128],
                                         slab[:, j, 0:n],
                                         start=(j == 0), stop=(j == 7))
                    nc.vector.tensor_copy(pos_t[:, G + c0:G + c0 + n],
                                          psp[:, 0:n])
                    nc.vector.tensor_copy(neg_t[:, G + c0:G + c0 + n],
                                          psn[:, 0:n])
                if s == 0:
                    dbg_dump('pos64', pos_t)
                sf = wpool.tile([128, 1152], bf16, tag="w1152")
                nc.sync.dma_start(out=sf[:], in_=ins[f'cmp{s}_sf'])
                sp_ = wpool.tile([128, 1152], bf16, tag="w1152")
                nc.sync.dma_start(out=sp_[:], in_=ins[f'cmp{s}_sp'])
                sn_ = wpool.tile([128, 1152], bf16, tag="w1152")
                nc.sync.dma_start(out=sn_[:], in_=ins[f'cmp{s}_sn'])
                b1 = load_bias(f'cmp{s}_b1')
                c1o = []
                for g in range(2):
                    dst = fresh_tile(acts, R, "act")
                    parts = [(f_t, 64 * g, 64, sf, 0, 128, 0),
                             (pos_t, 64 * g, 64, sp_, 0, 128, 0),
                             (neg_t, 64 * g, 64, sn_, 0, 128, 0)]
                    emit_conv(parts, dst, R, b1)
                    c1o.append(dst)
                if s == 0:
                    dbg_dump('c1o64', c1o[0])
                s2 = load_w(f'cmp{s}_s2')
                b2 = load_bias(f'cmp{s}_b2')
                c2o = []
                for g in range(2):
                    dst = fresh_tile(acts, R, "act")
                    emit_conv([(c1o[g], 0, 128, s2, 0, 128, 0)], dst, R, b2)
                    c2o.append(dst)
                s3 = wpool.tile([128, 576], bf16, tag="w1152")
                nc.sync.dma_start(out=s3[:], in_=ins[f'cmp{s}_s3'])
                b3 = load_bias(f'cmp{s}_b3')
                dsin = fresh_tile(acts, R, "act")
                for (c0, n) in px_chunks(PX):
                    ps = pspool.tile([128, 512], f32, tag="ps")
                    for g in range(2):
                        for t in range(9):
                            off = (t // 3 - 1) * W + (t % 3 - 1)
                            rhs = c2o[g][0:128, G + c0 + off:G + c0 + off + n]
                            lhsT = s3[0:128, t * 64:t * 64 + 64]
                            nc.tensor.matmul(ps[64 * g:64 * g + 64, 0:n],
                                             lhsT, rhs,
                                             start=(t == 0), stop=(t == 8),
                                             tile_position=(0, 64 * g))
                    nc.scalar.activation(out=dsin[:, G + c0:G + c0 + n],
                                         in_=ps[:, 0:n], func=LRELU,
                                         bias=b3[:, 0:1], alpha=0.1)
                zero_padcols(dsin, R)
                sds = load_w(f'stat_ds{s}')
                bds = load_bias(f'bias_ds{s}')
                fnew = fresh_tile(fpool, R // 2, "f")
                emit_ds(dsin, fnew, R, sds, bds)
                f_t = fnew
                if s == 0:
                    dbg_dump('ds64', f_t)

            # ---------------- decoder ----------------
            W8, G8, PX8 = res_params(8)
            with tc.tile_pool(name="dec", bufs=1) as decp:
                dec_in = decp.tile([16, DEC_IN_L], bf16, tag="dec_in")
                nc.vector.memset(dec_in[:], 0.0)
                for n in range(8):
                    nc.sync.dma_start(
                        out=dec_in[:, DEC_LEAD + NS1 * n:
                                   DEC_LEAD + NS1 * n + 80],
                        in_=f_t[16 * n:16 * n + 16, G8:G8 + 80])
                sd1 = decp.tile([16, 2304], bf16, tag="sd1")
                nc.sync.dma_start(out=sd1[:], in_=ins['stat_dec1'])
                bd1 = spool.tile([128, 2], f32, tag="bias")
                nc.sync.dma_start(out=bd1[:], in_=ins['bias_dec1'])
                d1 = []
                for h in range(2):
                    d1t = decp.tile([128, D1_L], bf16, tag=f"d1_{h}")
                    nc.vector.memset(d1t[:], 0.0)
                    ps = pspool.tile([128, 512], f32, tag="ps")
                    dia = dec_in[:]
                    for t in range(9):
                        dr, dc = t // 3 - 1, t % 3 - 1
                        base = DEC_LEAD + dr * 10 + dc
                        rhs = bass.AP(
                            tensor=dia.tensor, offset=dia.offset + base,
                            ap=[[dia.ap[0][0], 16], [NS1, 8], [20, 4], [2, 6]])
                        lhsT = sd1[0:16,
                                   (h * 9 + t) * 128:(h * 9 + t + 1) * 128]
                        nc.tensor.matmul(ps[0:128, 0:192],
                                         lhsT, rhs,
                                         start=(t == 0), stop=(t == 8))
                    oap = d1t[:]
                    out_ap = bass.AP(
                        tensor=oap.tensor, offset=oap.offset + D1_LEAD,
                        ap=[[oap.ap[0][0], 128], [NS2, 8], [6, 4], [1, 6]])
                    nc.scalar.activation(out=out_ap, in_=ps[:, 0:192],
                                         func=LRELU, bias=bd1[:, h:h + 1],
                                         alpha=0.1)
                    pad_ap = bass.AP(
                        tensor=oap.tensor, offset=oap.offset + D1_LEAD + 4,
                        ap=[[oap.ap[0][0], 128], [NS2, 8], [6, 4], [1, 2]])
                    nc.vector.memset(pad_ap, 0.0)
                    d1.append(d1t)
                sd2 = decp.tile([128, 2304], bf16, tag="sd2")
                nc.sync.dma_start(out=sd2[:], in_=ins['stat_dec2'])
                bd2 = load_bias('bias_dec2')
                d2 = decp.tile([128, D23_L], bf16, tag="d2")
                nc.vector.memset(d2[:], 0.0)
                ps2 = pspool.tile([128, 512], f32, tag="ps")
                i = 0
                for h in range(2):
                    dia = d1[h][:]
                    for t in range(9):
                        dr, dc = t // 3 - 1, t % 3 - 1
                        base = D1_LEAD + dr * 6 + dc
                        rhs = bass.AP(
                            tensor=dia.tensor, offset=dia.offset + base,
                            ap=[[dia.ap[0][0], 128], [NS2, 8], [12, 2], [2, 4]])
                        lhsT = sd2[0:128,
                                   (h * 9 + t) * 128:(h * 9 + t + 1) * 128]
                        i += 1
                        nc.tensor.matmul(ps2[0:128, 0:64],
                                         lhsT, rhs,
                                         start=(i == 1), stop=(i == 18))
                oap = d2[:]
                out_ap = bass.AP(tensor=oap.tensor,
                                 offset=oap.offset + D1_LEAD,
                                 ap=[[oap.ap[0][0], 128], [NS3, 8], [4, 2],
                                     [1, 4]])
                nc.scalar.activation(out=out_ap, in_=ps2[:, 0:64], func=LRELU,
                                     bias=bd2[:, 0:1], alpha=0.1)
                pad_ap = bass.AP(tensor=oap.tensor,
                                 offset=oap.offset + D1_LEAD + 2,
                                 ap=[[oap.ap[0][0], 128], [NS3, 8], [4, 2],
                                     [1, 2]])
                nc.vector.memset(pad_ap, 0.0)
                sd3 = decp.tile([128, 1152], bf16, tag="sd3")
                nc.sync.dma_start(out=sd3[:], in_=ins['stat_dec3'])
                bd3 = load_bias('bias_dec3')
                d3 = decp.tile([128, D23_L], bf16, tag="d3")
                nc.vector.memset(d3[:], 0.0)
                ps3 = pspool.tile([128, 512], f32, tag="ps")
                dia = d2[:]
                for t in range(9):
                    dr, dc = t // 3 - 1, t % 3 - 1
                    base = D1_LEAD + dr * 4 + dc
                    rhs = bass.AP(
                        tensor=dia.tensor, offset=dia.offset + base,
                        ap=[[dia.ap[0][0], 128], [NS3, 8], [4, 2], [1, 2]])
                    lhsT = sd3[0:128, t * 128:(t + 1) * 128]
                    nc.tensor.matmul(ps3[0:128, 0:32],
                                     lhsT, rhs,
                                     start=(t == 0), stop=(t == 8))
                oap = d3[:]
                out_ap = bass.AP(tensor=oap.tensor,
                                 offset=oap.offset + D1_LEAD,
                                 ap=[[oap.ap[0][0], 128], [NS3, 8], [4, 2],
                                     [1, 2]])
                nc.scalar.activation(out=out_ap, in_=ps3[:, 0:32], func=LRELU,
                                     bias=bd3[:, 0:1], alpha=0.1)
                hs = decp.tile([128, 16], bf16, tag="hs")
                nc.sync.dma_start(out=hs[:], in_=ins['head_stat'])
                psh = pshp.tile([4, 8], f32, tag="psh")
                dia = d3[:]
                for hwi, (r, c) in enumerate(((0, 0), (0, 1), (1, 0), (1, 1))):
                    rhs = bass.AP(
                        tensor=dia.tensor,
                        offset=dia.offset + D1_LEAD + r * 4 + c,
                        ap=[[dia.ap[0][0], 128], [NS3, 8]])
                    lhsT = hs[0:128, hwi * 4:hwi * 4 + 4]
                    nc.tensor.matmul(psh[0:4, 0:8],
                                     lhsT, rhs,
                                     start=(hwi == 0), stop=(hwi == 3))
                hout = decp.tile([4, 8], f32, tag="hout")
                nc.vector.tensor_copy(hout[:], psh[:])
                nc.sync.dma_start(out=head_d, in_=hout[:])

    nc.compile()
    return nc


# ---------------------------------------------------------------------------
# runner
# ---------------------------------------------------------------------------

_CACHED = {}


def kernel(x, given_y, given_w, nd_to_sample, params, _debug_taps=()):
    from concourse.bass_utils import run_bass_kernel_spmd

    key = tuple(_debug_taps)
    if key not in _CACHED:
        _CACHED[key] = build_program(debug_taps=_debug_taps)
    nc = _CACHED[key]

    in_maps, host_ctx = prep_host(x, given_y, given_w, nd_to_sample, params)
    res = run_bass_kernel_spmd(nc, in_maps, core_ids=list(range(N_CORES)))
    heads = [res.results[i]["head_part"] for i in range(N_CORES)]
    out = finish_host(heads, host_ctx)
    if _debug_taps:
        dbgs = [{k: res.results[i][f"dbg_{k}"] for k in _debug_taps}
                for i in range(N_CORES)]
        return out, dbgs
    return out
